# revision 21
# baseline (speedup 1.0000x reference)
"""Single-head attention kernel for Trainium2 (Bass/Tile), 8-core data-parallel.

Problem: h [8, 4096, 96] f32; Wq/Wk/Wv [96, 96]; bq/bk/bv [96].
  Q = h @ Wq.T + bq ; K = h @ Wk.T + bk ; V = h @ Wv.T + bv
  out = softmax(Q K^T / sqrt(96)) @ V

Sharding: batch dim across the 8 NeuronCores (1 batch element per core),
params replicated. Each core runs a flash-style attention over its
[4096, 96] slice; full output gathered on host.

Per-core structure (B=1, S=4096, D=96), fused-projection formulation:
  scores^T_j = h~_j M h~^T   with  M = W~k W~q^T / sqrt(D)  (97x97,
    augmented with bias row+col; M^T built directly from the DMA'd
    weight layouts with two matmuls, no transposes)
  U = M h~^T [97, S] bf16 replaces the Q/K projections entirely:
    scores_j = matmul(lhsT=h~^T_j, rhs=U).
  out^T = W~v^T (h~8^T e8) : PV accumulates Macc = h~8^T e8 with RAW h
    in fp8 e4m3 as the stationary operand (DoubleRow, j-tile pairs,
    contraction 256 = 2 rows/cycle), then W~v is applied once per sweep
    (2 matmuls). No V projection. Macc row 96 = softmax denominators
    (ones column of h~8).
  exp: softmax is shift-invariant; exp(s - 3) keeps e8 within e4m3
    range. ACT computes exp with bias=-3 writing fp8 directly; an
    interleaved subset of j tiles runs a single-op Schraudolph on DVE:
    u8 = sat(round(A8*s + B8)) bitcast to e4m3 (uint8 conversion
    saturates at 0 on HW, clamping underflow to +0.0).
  h casts (f32 -> bf16 for transposes/U, f32 -> fp8 for PV) run on the
    otherwise-idle GpSimd engine (SBUF->SBUF).
  PSUM: 3 rotating [128,1024] score slots (banks 0-5) + one 2-bank slot
    (banks 6-7) alternating between Macc and the per-sweep W~v-apply.
  Epilogue per sweep: Macc -> oT bf16 (DVE), W~v apply (PE), oV bf16
    copies (DVE), then 8 units: PE-transpose oV chunk + denom chunk,
    DVE reciprocal + scale, one batched store DMA per half-sweep.
  End-to-end rel err ~1.3e-2 against the 2e-2 gate (fp8 dominates).
"""

import functools
import math

import numpy as np

import concourse.mybir as mybir
import concourse.tile as tile
from concourse import bacc
from concourse.bass import ts
from concourse.bass_utils import run_bass_kernel_spmd

S = 4096
D = 96
P = 128              # s-tile (partition) size
N_CORES = 8
F32 = mybir.dt.float32
F32R = mybir.dt.float32r
BF16 = mybir.dt.bfloat16
FP8 = mybir.dt.float8e4
U8 = mybir.dt.uint8
AF = mybir.ActivationFunctionType
DROW = mybir.MatmulPerfMode.DoubleRow

# exp shift: softmax(s) == softmax(s - C); C=3 keeps exp(s-C) within the
# e4m3 range (max logit ~6.6 -> e^3.6 ~ 36 << 240) with headroom, while
# tails below e^-9ish flush to zero (negligible softmax mass).
EXP_SHIFT = 3.0
# Single-op fp8 Schraudolph on DVE: u8 = sat_u8(round(A8*s + B8)); the
# u8 bit pattern read as e4m3 approximates exp(s - EXP_SHIFT). 0.4639
# centers the mantissa-linear sawtooth. Conversion rounds-to-nearest and
# saturates [0, 255] on HW (probed), so negative logits clamp to +0.0.
SCH_A8 = 8.0 / math.log(2)
SCH_B8 = 56.0 - SCH_A8 * EXP_SHIFT - 0.4639
# j tiles whose exp runs on DVE instead of ACT, spread at single-j
# granularity so ACT (which drains its queue in order) never idles
# waiting on a DVE tile to release a PSUM slot. Sweep 0 offloads only
# later js (DVE carries the h~^T / U prologue copies early on).
OFF_JS_STEADY = frozenset(range(6, 32, 2))      # 13 js, sweeps 1-3
# sweep 0: sparse early (DVE still carries h~^T/U prologue copies),
# dense evens once the transposes are done (~g12)
OFF_JS_SWEEP0 = frozenset((4, 8)) | frozenset(range(14, 32, 2))


def _is_off_g(g):
    j = g & 31
    if g < 32:
        return j in OFF_JS_SWEEP0
    return j in OFF_JS_STEADY


def _is_off_pair(p):
    g0 = 2 * p
    return _is_off_g(g0) or _is_off_g(g0 + 1)


def build_attention_kernel(tc, out_dram, h, Wq, bq, Wk, bk, Wv, bv, s=S):
    nc = tc.nc
    nj = s // P            # 32 j tiles (K/V position tiles)
    nsw = s // 1024        # 4 i-sweeps of 1024 columns
    G = nsw * nj           # 128 global iterations
    scale = 1.0 / math.sqrt(D)

    from contextlib import ExitStack
    with ExitStack() as ctx:
        singles = ctx.enter_context(tc.tile_pool(name="singles", bufs=1))
        tmp = ctx.enter_context(tc.tile_pool(name="tmp", bufs=8))
        expp = ctx.enter_context(tc.tile_pool(name="expp", bufs=7))
        epi = ctx.enter_context(tc.tile_pool(name="epi", bufs=2))
        outp = ctx.enter_context(tc.tile_pool(name="outp", bufs=2))
        osbp = ctx.enter_context(tc.tile_pool(name="osbp", bufs=2))
        # PSUM: 3 rotating slots (scores tiles and small transpose/
        # projection tiles share them -- separate tiles per slot keep the
        # dependency tracking fine-grained) + the Macc/wv-apply slot.
        psp = ctx.enter_context(
            tc.tile_pool(name="psp", bufs=3, space="PSUM"))
        ps_accp = ctx.enter_context(
            tc.tile_pool(name="ps_acc", bufs=1, space="PSUM"))

        ident_dram = nc.inline_tensor(np.eye(P, dtype=np.float32),
                                      name="ident_const")

        # --- persistent SBUF tensors ---
        h_sb = singles.tile([P, nj, D], F32)      # staged h (row-major tiles)
        h_bf = singles.tile([P, nj, D], BF16)     # bf16 copy (GpSimd cast)
        # h~ fp8 (col 96 = ones); inner dim padded to 112 so the DoubleRow
        # pair stride is 16B-aligned (dual-fp8 LDWEIGHTS ISA restriction).
        VP = 112
        h8 = singles.tile([P, nj, VP], FP8)
        hT = singles.tile([D + 1, s], BF16)       # h~^T (row 96 = ones)
        UT = singles.tile([D + 1, s], BF16)       # U = M h~^T
        MT_sb = singles.tile([D + 1, D + 1], BF16)
        ident = singles.tile([P, P], F32)

        # --- prologue DMAs ---
        # ident first (transposes need it immediately), then h in 5 DMAs
        # on the sync HWDGE queue: dst[p, t, e] = h[(8k+t)*128 + p, e].
        src0 = h[0:512, :].rearrange("(t p) e -> p t e", p=P)
        nc.sync.dma_start(out=h_sb[:, 0:4, :], in_=src0)
        nc.sync.dma_start(out=ident, in_=ident_dram.ap())
        src1 = h[512:1024, :].rearrange("(t p) e -> p t e", p=P)
        nc.sync.dma_start(out=h_sb[:, 4:8, :], in_=src1)
        for k in range(1, 4):
            src = h[k * 1024:(k + 1) * 1024, :].rearrange(
                "(t p) e -> p t e", p=P)
            nc.sync.dma_start(out=h_sb[:, 8 * k:8 * k + 8, :], in_=src)
        # weights on the scalar HWDGE queue. Wq/Wk/biases first: the M
        # matmuls sit on the critical chain to U chunk 0 / first scores.
        wk_aug = tmp.tile([D, D + 1], F32)        # [Wk | bk]
        w_sb_q = tmp.tile([D, D], F32)
        bq_col = tmp.tile([D, 1], F32)
        nc.scalar.dma_start(out=w_sb_q, in_=Wq)
        nc.scalar.dma_start(out=wk_aug[:, 0:D], in_=Wk)
        nc.scalar.dma_start(out=wk_aug[:, D:D + 1], in_=bk.unsqueeze(1))
        nc.scalar.dma_start(out=bq_col, in_=bq.unsqueeze(1))
        w_sb_v = tmp.tile([D, D], F32)
        b_sb_v = tmp.tile([1, D], F32)
        nc.scalar.dma_start(out=w_sb_v, in_=Wv)
        nc.scalar.dma_start(out=b_sb_v, in_=bv.unsqueeze(0))
        # 1-partition ones-row memsets on DVE: they hide in the h-DMA
        # shadow (DVE is otherwise idle until the first h~^T copies),
        # and would serialize the GpSimd cast queue for multiple us.
        exp_bias = singles.tile([P, 1], F32)
        nc.vector.memset(hT[D:D + 1, 0:1024], 1.0)
        nc.vector.memset(hT[D:D + 1, 1024:s], 1.0)
        nc.vector.memset(exp_bias, -EXP_SHIFT)
        # h bf16 casts on the Pool engine (SBUF->SBUF), chasing the h
        # DMAs; the full-tile h8 memset (ones column = untouched pad and
        # col 96 after the fp8 casts overwrite cols 0:95) is ordered
        # after the first three bf16 chunks the transposes need next.
        nc.gpsimd.tensor_copy(h_bf[:, 0:4, :], h_sb[:, 0:4, :])
        nc.gpsimd.tensor_copy(h_bf[:, 4:8, :], h_sb[:, 4:8, :])
        nc.gpsimd.tensor_copy(h_bf[:, 8:16, :], h_sb[:, 8:16, :])
        nc.gpsimd.memset(h8, 1.0)

        # --- M^T = (W~q W~k^T) * scale, no transposes needed:
        # MT[m,n] = sum_e Wq[e,m] W~k[n,e] -> lhsT=w_sb_q, rhs=wk_aug;
        # row 96 (bias-of-q) via lhsT=bq_col.
        ps_m = psp.tile([D, D + 1], F32, tag="ps")
        nc.tensor.matmul(ps_m, lhsT=w_sb_q, rhs=wk_aug,
                         start=True, stop=True)
        ps_mb = psp.tile([1, D + 1], F32, tag="ps")
        nc.tensor.matmul(ps_mb, lhsT=bq_col, rhs=wk_aug,
                         start=True, stop=True)
        nc.vector.tensor_scalar_mul(MT_sb[0:D, :], ps_m, scale)
        nc.vector.tensor_scalar_mul(MT_sb[D:D + 1, :], ps_mb, scale)

        ident_bf = singles.tile([P, P], BF16)
        nc.vector.tensor_copy(ident_bf, ident)

        # --- augmented V weight W~v [97, 97] bf16: row 96 = bias, col 96
        # = e_96 so the wv-apply matmul passes the Macc denominator row
        # through into wv_ps (keeps the epilogue a single transpose).
        def build_wvt():
            ps_w = psp.tile([D, D], F32, tag="ps")
            nc.tensor.transpose(ps_w, w_sb_v, ident[0:D, 0:D])
            wt = singles.tile([D + 1, D + 1], BF16, name="wvt")
            nc.gpsimd.memset(wt[0:D, D:D + 1], 0.0)
            nc.gpsimd.memset(wt[D:D + 1, D:D + 1], 1.0)
            nc.vector.tensor_copy(wt[0:D, 0:D], ps_w)
            nc.vector.tensor_copy(wt[D:D + 1, 0:D], b_sb_v)
            return wt

        # --- emission helpers ---
        def emit_transpose(j):
            ps_t = psp.tile([D, P], BF16, tag="ps")
            nc.tensor.transpose(ps_t, h_bf[:, j, :], ident_bf)
            nc.vector.tensor_copy(hT[0:D, ts(j, P)], ps_t)

        def emit_ut(n):
            ps_u = psp.tile([D + 1, 512], F32, tag="ps")
            nc.tensor.matmul(ps_u, lhsT=MT_sb, rhs=hT[:, ts(n, 512)],
                             start=True, stop=True)
            nc.vector.tensor_copy(UT[:, ts(n, 512)], ps_u)

        # --- prologue compute: minimum for g=0, rest interleaved ---
        for j in range(4):
            emit_transpose(j)
        emit_ut(0)
        for j in range(4, 8):
            emit_transpose(j)
        emit_ut(1)
        wvt = build_wvt()
        state = {"t": 8, "ut": 2}

        def extras(g):
            # stage the remaining h bf16 casts now that their DMAs are in
            if g in (0, 2):
                k = g // 2 + 2
                nc.gpsimd.tensor_copy(h_bf[:, 8 * k:8 * k + 8, :],
                                      h_sb[:, 8 * k:8 * k + 8, :])
            # fp8 casts on DVE mid-sweep-0: PV needs chunk k only after
            # pair 4k's emission slot, and GpSimd is busy with bf16 then
            if g in (6, 8, 10, 12):
                k = (g - 6) // 2
                nc.vector.tensor_copy(h8[:, 8 * k:8 * k + 8, 0:D],
                                      h_sb[:, 8 * k:8 * k + 8, :])
            for _ in range(2):
                if state["t"] < nj:
                    emit_transpose(state["t"])
                    state["t"] += 1
            if state["ut"] < 8 and state["t"] >= 4 * state["ut"] + 4:
                emit_ut(state["ut"])
                state["ut"] += 1

        # --- scores + exp (fp8 pair tiles) ---
        pair_tiles = [None] * (G // 2)

        def scores_of(g):
            sw, j = g >> 5, g & 31
            i0 = sw * 1024
            ps_s = psp.tile([P, 1024], F32, tag="ps")
            for n in range(2):
                nc.tensor.matmul(
                    ps_s[:, ts(n, 512)],
                    lhsT=hT[:, ts(j, P)],
                    rhs=UT[:, i0 + 512 * n: i0 + 512 * (n + 1)],
                    start=True, stop=True)
            p = g >> 1
            if (g & 1) == 0:
                pair_tiles[p] = expp.tile([P, 2, 1024], FP8, tag="exp",
                                          name="e8")
            half = pair_tiles[p][:, g & 1, :]
            if _is_off_g(g):
                nc.vector.tensor_scalar(
                    half.bitcast(U8), ps_s, SCH_A8, SCH_B8,
                    mybir.AluOpType.mult, mybir.AluOpType.add)
            else:
                nc.scalar.activation(out=half, in_=ps_s, func=AF.Exp,
                                     bias=exp_bias)

        # --- epilogue machinery ---
        def emit_acc_copy(oT, half, acc):
            nc.vector.tensor_copy(oT[:, ts(half, 512)],
                                  acc[:, ts(half, 512)])

        def emit_epilogue_unit(oV, c, o_sb):
            ps_tr = psp.tile([P, D + 1], BF16, tag="ps")
            nc.tensor.transpose(ps_tr, oV[:, ts(c, P)],
                                ident_bf[0:D + 1, 0:D + 1])
            rec = outp.tile([P, 1], F32, tag="rec")
            nc.vector.reciprocal(rec, ps_tr[:, D:D + 1])
            # the scale-mul runs on ACT (Copy activation with per-
            # partition scale): keeps sweep-boundary DVE free for exps
            nc.scalar.activation(out=o_sb[:, c, :], in_=ps_tr[:, 0:D],
                                 func=AF.Copy, scale=rec)

        def emit_out_dma(sw, o_sb, half):
            r0 = sw * 1024 + half * 512
            dst = out_dram[r0:r0 + 512, :].rearrange("(u p) e -> p u e", p=P)
            nc.sync.dma_start(out=dst, in_=o_sb[:, 4 * half:4 * half + 4, :])

        # --- PV pair emission (DoubleRow fp8, Macc = h~8^T e8) ---
        acc_of = {}
        wv_of = {}
        emitted = {}     # sweep -> pairs emitted
        postponed = {}   # sweep -> pairs awaiting emission (uniform lag)
        pending = []     # deferred epilogue closures

        def get_acc(sw):
            if sw not in acc_of:
                acc_of[sw] = ps_accp.tile([D + 1, 1024], F32, tag="acc",
                                          name="acc")
            return acc_of[sw]

        def emit_pair(p):
            sw = p >> 4
            cnt = emitted.get(sw, 0)
            j0 = 2 * (p & 15)
            e8 = pair_tiles[p]
            acc = get_acc(sw)
            for n in range(2):
                nc.tensor.matmul(acc[:, ts(n, 512)],
                                 lhsT=h8[:, j0:j0 + 2, 0:D + 1],
                                 rhs=e8[:, :, ts(n, 512)],
                                 start=(cnt == 0), stop=(cnt == 15),
                                 perf_mode=DROW)
            emitted[sw] = cnt + 1
            pair_tiles[p] = None

        def pv_slot(q):
            # every pair is postponed 2 slots: DVE-exp pairs get latency
            # slack, and the first pairs of a sweep wait out the wv_ps ->
            # acc PSUM handoff without blocking the PE stream.
            sw = q >> 4
            lst = postponed.setdefault(sw, [])
            lst.append(q)
            while lst and q - lst[0] >= 2:
                emit_pair(lst.pop(0))

        def finish_sweep(swd):
            for p in postponed.pop(swd, []):
                emit_pair(p)
            # sweep fully accumulated. Everything downstream goes through
            # the paced `pending` queue in small (<=700ns) pieces so
            # neither the PE's in-order stream nor the DVE exp stream
            # ever parks behind a multi-us epilogue burst (a PE stall
            # drops the p-state and halves matmul speed for ~3us).
            oT = epi.tile([D + 1, 1024], BF16, tag="oT")
            oV = epi.tile([D + 1, 1024], BF16, tag="oV")
            o_sb = osbp.tile([P, 8, D], F32, tag="o_sb")
            acc = acc_of.pop(swd)

            def do_wv_apply(n, t=oT):
                if n == 0:
                    wv_of[swd] = ps_accp.tile([D + 1, 1024], F32,
                                              tag="acc", name="wv_ps")
                nc.tensor.matmul(wv_of[swd][:, ts(n, 512)], lhsT=wvt,
                                 rhs=t[:, ts(n, 512)],
                                 start=True, stop=True)

            def do_ov_copy(n, v=oV):
                nc.vector.tensor_copy(v[:, ts(n, 512)],
                                      wv_of[swd][:, ts(n, 512)])
                if n == 1:
                    del wv_of[swd]

            pending.append(lambda: emit_acc_copy(oT, 0, acc))
            pending.append(lambda: emit_acc_copy(oT, 1, acc))
            pending.append(lambda: do_wv_apply(0))
            pending.append(lambda: do_ov_copy(0))
            pending.append(lambda: do_wv_apply(1))
            pending.append(lambda: do_ov_copy(1))
            for c in range(8):
                pending.append(
                    lambda v=oV, c=c, o=o_sb:
                    emit_epilogue_unit(v, c, o))
                if c == 3:
                    pending.append(
                        lambda sw=swd, o=o_sb: emit_out_dma(sw, o, 0))
            pending.append(
                lambda sw=swd, o=o_sb: emit_out_dma(sw, o, 1))

        # --- flat main loop ---
        LAG = 3
        for g in range(G):
            scores_of(g)
            extras(g)
            if pending and (g & 31) >= 2:
                pending.pop(0)()
                if pending and (g & 31) >= 6:
                    pending.pop(0)()
            gp = g - LAG
            if gp >= 0 and (gp & 1):
                pv_slot(gp >> 1)
                if (gp & 31) == 31:
                    finish_sweep(gp >> 5)

        # drain PV tail and remaining epilogues
        for gp in range(G - LAG, G):
            if gp & 1:
                pv_slot(gp >> 1)
                if (gp & 31) == 31:
                    finish_sweep(gp >> 5)
                    while pending:
                        pending.pop(0)()
        while pending:
            pending.pop(0)()


@functools.lru_cache(maxsize=None)
def _build_module(s=S):
    nc = bacc.Bacc("TRN2", target_bir_lowering=False, debug=False,
                   num_devices=N_CORES)
    h = nc.dram_tensor("h", [s, D], F32, kind="ExternalInput").ap()
    Wq = nc.dram_tensor("Wq", [D, D], F32, kind="ExternalInput").ap()
    bq = nc.dram_tensor("bq", [D], F32, kind="ExternalInput").ap()
    Wk = nc.dram_tensor("Wk", [D, D], F32, kind="ExternalInput").ap()
    bk = nc.dram_tensor("bk", [D], F32, kind="ExternalInput").ap()
    Wv = nc.dram_tensor("Wv", [D, D], F32, kind="ExternalInput").ap()
    bv = nc.dram_tensor("bv", [D], F32, kind="ExternalInput").ap()
    out = nc.dram_tensor("out", [s, D], F32, kind="ExternalOutput").ap()
    with tile.TileContext(nc) as tc:
        build_attention_kernel(tc, out, h, Wq, bq, Wk, bk, Wv, bv, s=s)
    nc.compile()
    return nc


def _run(inputs, trace=False):
    nc = _build_module(S)
    arrs = {k: np.ascontiguousarray(np.asarray(v), dtype=np.float32)
            for k, v in inputs.items()}
    in_maps = []
    for b_ in range(N_CORES):
        in_maps.append({
            "h": arrs["h"][b_],
            "Wq": arrs["Wq"], "bq": arrs["bq"],
            "Wk": arrs["Wk"], "bk": arrs["bk"],
            "Wv": arrs["Wv"], "bv": arrs["bv"],
        })
    res = run_bass_kernel_spmd(nc, in_maps, core_ids=list(range(N_CORES)),
                               trace=trace)
    out = np.stack([res.results[b_]["out"] for b_ in range(N_CORES)], axis=0)
    return out, res


def kernel(**inputs):
    out, _ = _run(inputs, trace=False)
    return out


def kernel_profiled(trace=True, **inputs):
    out, res = _run(inputs, trace=trace)
    return out, res


# revision 22
# speedup vs baseline: 1.0491x; 1.0491x over previous
"""Single-head attention kernel for Trainium2 (Bass/Tile), 8-core data-parallel.

Problem: h [8, 4096, 96] f32; Wq/Wk/Wv [96, 96]; bq/bk/bv [96].
  Q = h @ Wq.T + bq ; K = h @ Wk.T + bk ; V = h @ Wv.T + bv
  out = softmax(Q K^T / sqrt(96)) @ V

Sharding: batch dim across the 8 NeuronCores (1 batch element per core),
params replicated. Each core runs a flash-style attention over its
[4096, 96] slice; full output gathered on host.

Per-core structure (B=1, S=4096, D=96), fused-projection formulation:
  scores^T_j = h~_j M h~^T   with  M = W~k W~q^T / sqrt(D)  (97x97,
    augmented with bias row+col; M^T built directly from the DMA'd
    weight layouts with two matmuls, no transposes)
  U = M h~^T [97, S] bf16 replaces the Q/K projections entirely:
    scores_j = matmul(lhsT=h~^T_j, rhs=U).
  out^T = W~v^T (h~8^T e8) : PV accumulates Macc = h~8^T e8 with RAW h
    in fp8 e4m3 as the stationary operand (DoubleRow, j-tile pairs,
    contraction 256 = 2 rows/cycle), then W~v is applied once per sweep
    (2 matmuls). No V projection. Macc row 96 = softmax denominators
    (ones column of h~8).
  exp: softmax is shift-invariant; exp(s - 3) keeps e8 within e4m3
    range. ACT computes exp with bias=-3 writing fp8 directly; an
    interleaved subset of j tiles runs a single-op Schraudolph on DVE:
    u8 = sat(round(A8*s + B8)) bitcast to e4m3 (uint8 conversion
    saturates at 0 on HW, clamping underflow to +0.0).
  h casts (f32 -> bf16 for transposes/U, f32 -> fp8 for PV) run on the
    otherwise-idle GpSimd engine (SBUF->SBUF).
  PSUM: 3 rotating [128,1024] score slots (banks 0-5) + one 2-bank slot
    (banks 6-7) alternating between Macc and the per-sweep W~v-apply.
  Epilogue per sweep: Macc -> oT bf16 (DVE), W~v apply (PE), oV bf16
    copies (DVE), then 8 units: PE-transpose oV chunk + denom chunk,
    DVE reciprocal + scale, one batched store DMA per half-sweep.
  End-to-end rel err ~1.3e-2 against the 2e-2 gate (fp8 dominates).
"""

import functools
import math

import numpy as np

import concourse.mybir as mybir
import concourse.tile as tile
from concourse import bacc
from concourse.bass import ts
from concourse.bass_utils import run_bass_kernel_spmd

S = 4096
D = 96
P = 128              # s-tile (partition) size
N_CORES = 8
F32 = mybir.dt.float32
F32R = mybir.dt.float32r
BF16 = mybir.dt.bfloat16
FP8 = mybir.dt.float8e4
U8 = mybir.dt.uint8
AF = mybir.ActivationFunctionType
DROW = mybir.MatmulPerfMode.DoubleRow

# exp shift: softmax(s) == softmax(s - C); C=3 keeps exp(s-C) within the
# e4m3 range (max logit ~6.6 -> e^3.6 ~ 36 << 240) with headroom, while
# tails below e^-9ish flush to zero (negligible softmax mass).
EXP_SHIFT = 3.0
# Single-op fp8 Schraudolph on DVE: u8 = sat_u8(round(A8*s + B8)); the
# u8 bit pattern read as e4m3 approximates exp(s - EXP_SHIFT). 0.4639
# centers the mantissa-linear sawtooth. Conversion rounds-to-nearest and
# saturates [0, 255] on HW (probed), so negative logits clamp to +0.0.
SCH_A8 = 8.0 / math.log(2)
SCH_B8 = 56.0 - SCH_A8 * EXP_SHIFT - 0.4639
# j tiles whose exp runs on DVE instead of ACT, spread at single-j
# granularity so ACT (which drains its queue in order) never idles
# waiting on a DVE tile to release a PSUM slot. Sweep 0 offloads only
# later js (DVE carries the h~^T / U prologue copies early on).
OFF_JS_STEADY = frozenset(range(6, 32, 2))      # 13 js, sweeps 1-3
# sweep 0: sparse early (DVE still carries h~^T/U prologue copies),
# dense evens once the transposes are done (~g12)
OFF_JS_SWEEP0 = frozenset((4, 8)) | frozenset(range(14, 32, 2))


def _is_off_g(g):
    j = g & 31
    if g < 32:
        return j in OFF_JS_SWEEP0
    return j in OFF_JS_STEADY


def _is_off_pair(p):
    g0 = 2 * p
    return _is_off_g(g0) or _is_off_g(g0 + 1)


def build_attention_kernel(tc, out_dram, h, Wq, bq, Wk, bk, Wv, bv, s=S):
    nc = tc.nc
    nj = s // P            # 32 j tiles (K/V position tiles)
    nsw = s // 1024        # 4 i-sweeps of 1024 columns
    G = nsw * nj           # 128 global iterations
    scale = 1.0 / math.sqrt(D)

    from contextlib import ExitStack
    with ExitStack() as ctx:
        singles = ctx.enter_context(tc.tile_pool(name="singles", bufs=1))
        tmp = ctx.enter_context(tc.tile_pool(name="tmp", bufs=8))
        expp = ctx.enter_context(tc.tile_pool(name="expp", bufs=7))
        epi = ctx.enter_context(tc.tile_pool(name="epi", bufs=2))
        outp = ctx.enter_context(tc.tile_pool(name="outp", bufs=2))
        osbp = ctx.enter_context(tc.tile_pool(name="osbp", bufs=2))
        # PSUM: 3 rotating slots (scores tiles and small transpose/
        # projection tiles share them -- separate tiles per slot keep the
        # dependency tracking fine-grained) + the Macc/wv-apply slot.
        psp = ctx.enter_context(
            tc.tile_pool(name="psp", bufs=3, space="PSUM"))
        ps_accp = ctx.enter_context(
            tc.tile_pool(name="ps_acc", bufs=1, space="PSUM"))

        ident_dram = nc.inline_tensor(np.eye(P, dtype=np.float32),
                                      name="ident_const")

        # --- persistent SBUF tensors ---
        h_sb = singles.tile([P, nj, D], F32)      # staged h (row-major tiles)
        h_bf = singles.tile([P, nj, D], BF16)     # bf16 copy (GpSimd cast)
        # h~ fp8 (col 96 = ones); inner dim padded to 112 so the DoubleRow
        # pair stride is 16B-aligned (dual-fp8 LDWEIGHTS ISA restriction).
        VP = 112
        h8 = singles.tile([P, nj, VP], FP8)
        hT = singles.tile([D + 1, s], BF16)       # h~^T (row 96 = ones)
        UT = singles.tile([D + 1, s], BF16)       # U = M h~^T
        MT_sb = singles.tile([D + 1, D + 1], BF16)
        ident = singles.tile([P, P], F32)

        # --- prologue DMAs ---
        # ident first (transposes need it immediately), then h in 5 DMAs
        # on the sync HWDGE queue: dst[p, t, e] = h[(8k+t)*128 + p, e].
        src0 = h[0:512, :].rearrange("(t p) e -> p t e", p=P)
        nc.sync.dma_start(out=h_sb[:, 0:4, :], in_=src0)
        nc.sync.dma_start(out=ident, in_=ident_dram.ap())
        src1 = h[512:1024, :].rearrange("(t p) e -> p t e", p=P)
        nc.sync.dma_start(out=h_sb[:, 4:8, :], in_=src1)
        for k in range(1, 4):
            src = h[k * 1024:(k + 1) * 1024, :].rearrange(
                "(t p) e -> p t e", p=P)
            nc.sync.dma_start(out=h_sb[:, 8 * k:8 * k + 8, :], in_=src)
        # weights on the scalar HWDGE queue. Wq/Wk/biases first: the M
        # matmuls sit on the critical chain to U chunk 0 / first scores.
        wk_aug = tmp.tile([D, D + 1], F32)        # [Wk | bk]
        w_sb_q = tmp.tile([D, D], F32)
        bq_col = tmp.tile([D, 1], F32)
        nc.scalar.dma_start(out=w_sb_q, in_=Wq)
        nc.scalar.dma_start(out=wk_aug[:, 0:D], in_=Wk)
        nc.scalar.dma_start(out=wk_aug[:, D:D + 1], in_=bk.unsqueeze(1))
        nc.scalar.dma_start(out=bq_col, in_=bq.unsqueeze(1))
        w_sb_v = tmp.tile([D, D], F32)
        b_sb_v = tmp.tile([1, D], F32)
        nc.scalar.dma_start(out=w_sb_v, in_=Wv)
        nc.scalar.dma_start(out=b_sb_v, in_=bv.unsqueeze(0))
        # 1-partition ones-row memsets on DVE: they hide in the h-DMA
        # shadow (DVE is otherwise idle until the first h~^T copies),
        # and would serialize the GpSimd cast queue for multiple us.
        exp_bias = singles.tile([P, 1], F32)
        nc.vector.memset(hT[D:D + 1, 0:1024], 1.0)
        nc.vector.memset(hT[D:D + 1, 1024:s], 1.0)
        nc.vector.memset(exp_bias, -EXP_SHIFT)
        # h bf16 casts on the Pool engine (SBUF->SBUF), chasing the h
        # DMAs; the full-tile h8 memset (ones column = untouched pad and
        # col 96 after the fp8 casts overwrite cols 0:95) is ordered
        # after the first three bf16 chunks the transposes need next.
        nc.gpsimd.tensor_copy(h_bf[:, 0:4, :], h_sb[:, 0:4, :])
        nc.gpsimd.tensor_copy(h_bf[:, 4:8, :], h_sb[:, 4:8, :])
        nc.gpsimd.tensor_copy(h_bf[:, 8:16, :], h_sb[:, 8:16, :])
        nc.gpsimd.memset(h8, 1.0)

        # --- M^T = (W~q W~k^T) * scale, no transposes needed:
        # MT[m,n] = sum_e Wq[e,m] W~k[n,e] -> lhsT=w_sb_q, rhs=wk_aug;
        # row 96 (bias-of-q) via lhsT=bq_col.
        ps_m = psp.tile([D, D + 1], F32, tag="ps")
        nc.tensor.matmul(ps_m, lhsT=w_sb_q, rhs=wk_aug,
                         start=True, stop=True)
        ps_mb = psp.tile([1, D + 1], F32, tag="ps")
        nc.tensor.matmul(ps_mb, lhsT=bq_col, rhs=wk_aug,
                         start=True, stop=True)
        nc.vector.tensor_scalar_mul(MT_sb[0:D, :], ps_m, scale)
        nc.vector.tensor_scalar_mul(MT_sb[D:D + 1, :], ps_mb, scale)

        ident_bf = singles.tile([P, P], BF16)
        nc.vector.tensor_copy(ident_bf, ident)

        # --- augmented V weight W~v [97, 97] bf16: row 96 = bias, col 96
        # = e_96 so the wv-apply matmul passes the Macc denominator row
        # through into wv_ps (keeps the epilogue a single transpose).
        def build_wvt():
            ps_w = psp.tile([D, D], F32, tag="ps")
            nc.tensor.transpose(ps_w, w_sb_v, ident[0:D, 0:D])
            wt = singles.tile([D + 1, D + 1], BF16, name="wvt")
            nc.gpsimd.memset(wt[0:D, D:D + 1], 0.0)
            nc.gpsimd.memset(wt[D:D + 1, D:D + 1], 1.0)
            nc.vector.tensor_copy(wt[0:D, 0:D], ps_w)
            nc.vector.tensor_copy(wt[D:D + 1, 0:D], b_sb_v)
            return wt

        # --- emission helpers ---
        def emit_transpose(j):
            ps_t = psp.tile([D, P], BF16, tag="ps")
            nc.tensor.transpose(ps_t, h_bf[:, j, :], ident_bf)
            nc.vector.tensor_copy(hT[0:D, ts(j, P)], ps_t)

        def emit_ut(n):
            ps_u = psp.tile([D + 1, 512], F32, tag="ps")
            nc.tensor.matmul(ps_u, lhsT=MT_sb, rhs=hT[:, ts(n, 512)],
                             start=True, stop=True)
            nc.vector.tensor_copy(UT[:, ts(n, 512)], ps_u)

        # --- prologue compute: minimum for g=0, rest interleaved ---
        for j in range(4):
            emit_transpose(j)
        emit_ut(0)
        for j in range(4, 8):
            emit_transpose(j)
        emit_ut(1)
        wvt = build_wvt()
        state = {"t": 8, "ut": 2}

        def extras(g):
            # stage the remaining h bf16 casts now that their DMAs are in
            if g in (0, 2):
                k = g // 2 + 2
                nc.gpsimd.tensor_copy(h_bf[:, 8 * k:8 * k + 8, :],
                                      h_sb[:, 8 * k:8 * k + 8, :])
            # fp8 casts on DVE mid-sweep-0: PV needs chunk k only after
            # pair 4k's emission slot, and GpSimd is busy with bf16 then
            if g in (6, 8, 10, 12):
                k = (g - 6) // 2
                nc.vector.tensor_copy(h8[:, 8 * k:8 * k + 8, 0:D],
                                      h_sb[:, 8 * k:8 * k + 8, :])
            for _ in range(2):
                if state["t"] < nj:
                    emit_transpose(state["t"])
                    state["t"] += 1
            if state["ut"] < 8 and state["t"] >= 4 * state["ut"] + 4:
                emit_ut(state["ut"])
                state["ut"] += 1

        # --- scores + exp (fp8 pair tiles) ---
        pair_tiles = [None] * (G // 2)

        def scores_of(g):
            sw, j = g >> 5, g & 31
            i0 = sw * 1024
            ps_s = psp.tile([P, 1024], F32, tag="ps")
            for n in range(2):
                nc.tensor.matmul(
                    ps_s[:, ts(n, 512)],
                    lhsT=hT[:, ts(j, P)],
                    rhs=UT[:, i0 + 512 * n: i0 + 512 * (n + 1)],
                    start=True, stop=True)
            p = g >> 1
            if (g & 1) == 0:
                pair_tiles[p] = expp.tile([P, 2, 1024], FP8, tag="exp",
                                          name="e8")
            half = pair_tiles[p][:, g & 1, :]
            if _is_off_g(g):
                nc.vector.tensor_scalar(
                    half.bitcast(U8), ps_s, SCH_A8, SCH_B8,
                    mybir.AluOpType.mult, mybir.AluOpType.add)
            else:
                nc.scalar.activation(out=half, in_=ps_s, func=AF.Exp,
                                     bias=exp_bias)

        # --- epilogue machinery ---
        def emit_acc_copy(oT, half, acc):
            nc.vector.tensor_copy(oT[:, ts(half, 512)],
                                  acc[:, ts(half, 512)])

        def emit_epilogue_unit(oV, c, o_sb):
            ps_tr = psp.tile([P, D + 1], BF16, tag="ps")
            nc.tensor.transpose(ps_tr, oV[:, ts(c, P)],
                                ident_bf[0:D + 1, 0:D + 1])
            rec = outp.tile([P, 1], F32, tag="rec")
            nc.vector.reciprocal(rec, ps_tr[:, D:D + 1])
            nc.vector.tensor_scalar_mul(o_sb[:, c, :], ps_tr[:, 0:D], rec)

        def emit_out_dma(sw, o_sb, half):
            r0 = sw * 1024 + half * 512
            dst = out_dram[r0:r0 + 512, :].rearrange("(u p) e -> p u e", p=P)
            nc.sync.dma_start(out=dst, in_=o_sb[:, 4 * half:4 * half + 4, :])

        # --- PV pair emission (DoubleRow fp8, Macc = h~8^T e8) ---
        acc_of = {}
        wv_of = {}
        emitted = {}     # sweep -> pairs emitted
        postponed = {}   # sweep -> pairs awaiting emission (uniform lag)
        pending = []     # deferred epilogue closures

        def get_acc(sw):
            if sw not in acc_of:
                acc_of[sw] = ps_accp.tile([D + 1, 1024], F32, tag="acc",
                                          name="acc")
            return acc_of[sw]

        def emit_pair(p):
            sw = p >> 4
            cnt = emitted.get(sw, 0)
            j0 = 2 * (p & 15)
            e8 = pair_tiles[p]
            acc = get_acc(sw)
            for n in range(2):
                nc.tensor.matmul(acc[:, ts(n, 512)],
                                 lhsT=h8[:, j0:j0 + 2, 0:D + 1],
                                 rhs=e8[:, :, ts(n, 512)],
                                 start=(cnt == 0), stop=(cnt == 15),
                                 perf_mode=DROW)
            emitted[sw] = cnt + 1
            pair_tiles[p] = None

        def pv_slot(q):
            # every pair is postponed 2 slots: DVE-exp pairs get latency
            # slack, and the first pairs of a sweep wait out the wv_ps ->
            # acc PSUM handoff without blocking the PE stream.
            sw = q >> 4
            lst = postponed.setdefault(sw, [])
            lst.append(q)
            while lst and q - lst[0] >= 2:
                emit_pair(lst.pop(0))

        def finish_sweep(swd):
            for p in postponed.pop(swd, []):
                emit_pair(p)
            # sweep fully accumulated. Everything downstream goes through
            # the paced `pending` queue in small (<=700ns) pieces so
            # neither the PE's in-order stream nor the DVE exp stream
            # ever parks behind a multi-us epilogue burst (a PE stall
            # drops the p-state and halves matmul speed for ~3us).
            oT = epi.tile([D + 1, 1024], BF16, tag="oT")
            oV = epi.tile([D + 1, 1024], BF16, tag="oV")
            o_sb = osbp.tile([P, 8, D], F32, tag="o_sb")
            acc = acc_of.pop(swd)

            def do_wv_apply(n, t=oT):
                if n == 0:
                    wv_of[swd] = ps_accp.tile([D + 1, 1024], F32,
                                              tag="acc", name="wv_ps")
                nc.tensor.matmul(wv_of[swd][:, ts(n, 512)], lhsT=wvt,
                                 rhs=t[:, ts(n, 512)],
                                 start=True, stop=True)

            def do_ov_copy(n, v=oV):
                nc.vector.tensor_copy(v[:, ts(n, 512)],
                                      wv_of[swd][:, ts(n, 512)])
                if n == 1:
                    del wv_of[swd]

            pending.append(lambda: emit_acc_copy(oT, 0, acc))
            pending.append(lambda: emit_acc_copy(oT, 1, acc))
            pending.append(lambda: do_wv_apply(0))
            pending.append(lambda: do_ov_copy(0))
            pending.append(lambda: do_wv_apply(1))
            pending.append(lambda: do_ov_copy(1))
            for c in range(8):
                pending.append(
                    lambda v=oV, c=c, o=o_sb:
                    emit_epilogue_unit(v, c, o))
                if c == 3:
                    pending.append(
                        lambda sw=swd, o=o_sb: emit_out_dma(sw, o, 0))
            pending.append(
                lambda sw=swd, o=o_sb: emit_out_dma(sw, o, 1))

        # --- flat main loop ---
        LAG = 3
        for g in range(G):
            scores_of(g)
            extras(g)
            if pending and (g & 31) >= 2:
                pending.pop(0)()
                if pending and (g & 31) >= 6:
                    pending.pop(0)()
            gp = g - LAG
            if gp >= 0 and (gp & 1):
                pv_slot(gp >> 1)
                if (gp & 31) == 31:
                    finish_sweep(gp >> 5)

        # drain PV tail and remaining epilogues
        for gp in range(G - LAG, G):
            if gp & 1:
                pv_slot(gp >> 1)
                if (gp & 31) == 31:
                    finish_sweep(gp >> 5)
                    while pending:
                        pending.pop(0)()
        while pending:
            pending.pop(0)()


@functools.lru_cache(maxsize=None)
def _build_module(s=S):
    nc = bacc.Bacc("TRN2", target_bir_lowering=False, debug=False,
                   num_devices=N_CORES)
    h = nc.dram_tensor("h", [s, D], F32, kind="ExternalInput").ap()
    Wq = nc.dram_tensor("Wq", [D, D], F32, kind="ExternalInput").ap()
    bq = nc.dram_tensor("bq", [D], F32, kind="ExternalInput").ap()
    Wk = nc.dram_tensor("Wk", [D, D], F32, kind="ExternalInput").ap()
    bk = nc.dram_tensor("bk", [D], F32, kind="ExternalInput").ap()
    Wv = nc.dram_tensor("Wv", [D, D], F32, kind="ExternalInput").ap()
    bv = nc.dram_tensor("bv", [D], F32, kind="ExternalInput").ap()
    out = nc.dram_tensor("out", [s, D], F32, kind="ExternalOutput").ap()
    with tile.TileContext(nc) as tc:
        build_attention_kernel(tc, out, h, Wq, bq, Wk, bk, Wv, bv, s=s)
    nc.compile()
    return nc


def _run(inputs, trace=False):
    nc = _build_module(S)
    arrs = {k: np.ascontiguousarray(np.asarray(v), dtype=np.float32)
            for k, v in inputs.items()}
    in_maps = []
    for b_ in range(N_CORES):
        in_maps.append({
            "h": arrs["h"][b_],
            "Wq": arrs["Wq"], "bq": arrs["bq"],
            "Wk": arrs["Wk"], "bk": arrs["bk"],
            "Wv": arrs["Wv"], "bv": arrs["bv"],
        })
    res = run_bass_kernel_spmd(nc, in_maps, core_ids=list(range(N_CORES)),
                               trace=trace)
    out = np.stack([res.results[b_]["out"] for b_ in range(N_CORES)], axis=0)
    return out, res


def kernel(**inputs):
    out, _ = _run(inputs, trace=False)
    return out


def kernel_profiled(trace=True, **inputs):
    out, res = _run(inputs, trace=trace)
    return out, res
